# revision 38
# baseline (speedup 1.0000x reference)
"""Fused multi-LoRA linear layer on 8 TRN2 NeuronCores.

out = x @ W.T + b + scale * mask(x @ A_all^T) @ B_flat

Sharding: data-parallel over the token dim N (32768 -> 8 x 4096).
Weights (W, A_all, B_all, b) are replicated; each core computes its token
shard fully, so no collectives are needed.

v4: mixed-precision PE schedule + chunk-pair weight reuse.
- The LoRA down-projection and the first 256 contraction dims of the base
  matmul run as fp8e4 DoubleRow matmuls (2 contraction rows/cycle); the
  remaining 14 k-tiles stay bf16. All device math is scaled by GS = 2^16
  so fp8 and bf16 partial products share PSUM banks; the host descales.
- Token chunks are processed in pairs that share every stationary weight
  load (LDWEIGHTS gets a 2-matmul window to hide under).
- The first pair runs W-column-group-major so the PE starts ~2us after
  the first small DMAs land and each W group arrives before its quad.
"""

import numpy as np
import ml_dtypes

# Problem constants (hardcoded per harness contract).
N, D_IN, D_OUT, L, R = 32768, 2048, 2048, 8, 16
SCALE = 32.0 / 16.0
M_CORES = 8
NS = N // M_CORES  # 4096 tokens per core
P = 128
KT = D_IN // P  # 16 k-tiles
KQ = 2  # fp8 DoubleRow k-PAIRS used in the base matmul (k-tiles 0..2*KQ-1)
KB = KT - 2 * KQ  # bf16 k-tiles (k = 2*KQ .. 15)
KD = KT // 2  # fp8 k-pairs in the down-projection (all 8)
OI = D_OUT // P  # 16 output row-chunks of 128
TW = 512  # token tile width (moving free dim)
TC = NS // TW  # 8 token chunks per core
LR = L * R  # 128
WG = 4  # W column groups
WGC = D_OUT // WG  # 512 columns per group

SX = 32.0  # fp8 scale for x
SQ = 2048.0  # fp8 scale for W and A
GS = SX * SQ  # 2^16: global scale of all device-side math

_BF16 = ml_dtypes.bfloat16
_F8 = ml_dtypes.float8_e4m3

_CACHE = {}

LAST_EXEC_TIME_NS = None


def _build():
    import concourse.bass as bass  # noqa: F401
    import concourse.tile as tile
    from concourse import bacc, mybir
    from concourse.tile_rust import add_dep_helper
    from contextlib import ExitStack

    bf16 = mybir.dt.bfloat16
    f8 = mybir.dt.float8e4
    f32 = mybir.dt.float32
    DR = mybir.MatmulPerfMode.DoubleRow

    nc = bacc.Bacc(
        "TRN2",
        target_bir_lowering=False,
        debug=False,
        num_devices=M_CORES,
    )

    # Host-prepared, partition-major layouts (see kernel()):
    #   xT [TC, P, KB, TW] bf16 : xT[t,p,kb,j] = x[t*TW+j, (kb+2KQ)*P+p]
    #   xQ [TC, P, KD, 2, TW] f8: xQ[t,p,kk,u,j] = q8(x[t*TW+j, (2kk+u)*P+p]*SX)
    #   wT [WG, P, KB, WGC] bf16: wT[g,p,kb,o] = W[g*WGC+o, (kb+2KQ)*P+p]*GS
    #   wQ [P, KQ, 2, D_OUT] f8 : wQ[p,kk,u,o] = q8(W[o, (2kk+u)*P+p]*SQ)
    #   aQ [P, KD, 2, LR] f8    : aQ[p,kk,u,c] = q8(A_flat[c, (2kk+u)*P+p]*SQ)
    #   bF [P, D_OUT] bf16      : bF[c,o] = B_all[c//R, o, c%R]
    #   bias [P, OI] f32        : bias[p,oi] = b[oi*P+p]*GS
    #   mT [TC, P, TW] bf16     : one-hot adapter mask * SCALE
    xT = nc.dram_tensor("xT", [TC, P, KB, TW], bf16, kind="ExternalInput").ap()
    xQ = nc.dram_tensor("xQ", [TC, P, KD, 2, TW], f8, kind="ExternalInput").ap()
    wT = nc.dram_tensor("wT", [WG, P, KB, WGC], bf16, kind="ExternalInput").ap()
    wQ = nc.dram_tensor("wQ", [P, KQ, 2, D_OUT], f8, kind="ExternalInput").ap()
    aQ = nc.dram_tensor("aQ", [P, KD, 2, LR], f8, kind="ExternalInput").ap()
    bF = nc.dram_tensor("bF", [P, D_OUT], bf16, kind="ExternalInput").ap()
    bias = nc.dram_tensor("bias", [P, OI], f32, kind="ExternalInput").ap()
    mT = nc.dram_tensor("mT", [TC, P, TW], bf16, kind="ExternalInput").ap()
    outT = nc.dram_tensor("outT", [D_OUT, NS], f32, kind="ExternalOutput").ap()

    with tile.TileContext(nc) as tc, ExitStack() as ctx:
        warm_pool = ctx.enter_context(tc.tile_pool(name="warm", bufs=1))
        aq_pool = ctx.enter_context(tc.tile_pool(name="aq", bufs=1))
        wq_pool = ctx.enter_context(tc.tile_pool(name="wq", bufs=1))
        bf_pool = ctx.enter_context(tc.tile_pool(name="bfp", bufs=1))
        bias_pool = ctx.enter_context(tc.tile_pool(name="bias", bufs=1))
        mask_pool = ctx.enter_context(tc.tile_pool(name="mask", bufs=4))
        # chunk-0 slab-loaded input tiles (stay resident; the w0 tile serves
        # as the g=0 weight tile for every chunk)
        xq0_pool = ctx.enter_context(tc.tile_pool(name="xq0", bufs=1))
        x0_pool = ctx.enter_context(tc.tile_pool(name="x0", bufs=1))
        w0_pool = ctx.enter_context(tc.tile_pool(name="w0", bufs=1))
        wt_pool = ctx.enter_context(tc.tile_pool(name="wt", bufs=WG - 1))
        xq_pool = ctx.enter_context(tc.tile_pool(name="xq", bufs=4))
        x_pool = ctx.enter_context(tc.tile_pool(name="x", bufs=4))
        u_pool = ctx.enter_context(tc.tile_pool(name="u", bufs=2))
        o_pool = ctx.enter_context(tc.tile_pool(name="o", bufs=4))
        pw_pool = ctx.enter_context(tc.tile_pool(name="pw", bufs=1, space="PSUM"))
        pu_pool = ctx.enter_context(tc.tile_pool(name="pu", bufs=2, space="PSUM"))
        po_pool = ctx.enter_context(tc.tile_pool(name="po", bufs=4, space="PSUM"))

        # Short PE warmup: covers the first small DMAs and starts the HAM
        # clock ramp while chunk-0 slices stream in.
        warm = warm_pool.tile([P, P], bf16)
        nc.vector.memset(warm[:], 0.0)
        pw = pw_pool.tile([P, P], mybir.dt.float32)
        for _ in range(32):
            nc.tensor.matmul(pw[:], warm[:], warm[:], start=True, stop=True)

        def load_mask(t, gate=None):
            mk = mask_pool.tile([P, TW], bf16, tag="mk", name="mk")
            dma = nc.sync.dma_start(mk[:], mT[t])
            if gate is not None:
                add_dep_helper(dma.ins, gate.ins, sync=True, reason="pace")
            return mk

        # scalar ring head: A (tiny, needed by the very first matmul).
        aq = aq_pool.tile([P, KD, 2, LR], f8)
        nc.scalar.dma_start(aq[:], aQ[:, :, :, :])
        # sync ring: chunk-0 fp8 slabs first (the other first-matmul input).
        xq0_t = xq0_pool.tile([P, KD, 2, TW], f8)
        for lo, hi in ((0, 3), (3, 6), (6, KD)):
            nc.sync.dma_start(xq0_t[:, lo:hi, :, :], xQ[0, :, lo:hi, :, :])
        mk0 = load_mask(0)
        bias_t = bias_pool.tile([P, OI], f32)
        nc.sync.dma_start(bias_t[:], bias[:, :])
        x0_t = x0_pool.tile([P, KB, TW], bf16)
        for lo in range(0, KB, 3):
            hi = min(lo + 3, KB)
            nc.sync.dma_start(x0_t[:, lo:hi, :], xT[0, :, lo:hi, :])

        # scalar ring: fp8 base weights, then g0 slabs.
        wq = wq_pool.tile([P, KQ, 2, D_OUT], f8)
        nc.scalar.dma_start(wq[:], wQ[:, :, :, :])
        w0_t = w0_pool.tile([P, KB, WGC], bf16)
        for lo in range(0, KB, 3):
            hi = min(lo + 3, KB)
            nc.scalar.dma_start(w0_t[:, lo:hi, :], wT[0, :, lo:hi, :])
        bf_t = bf_pool.tile([P, D_OUT], bf16)
        bf_dma = nc.scalar.dma_start(bf_t[:], bF[:, :])
        # g1..g3 ride the idle GpSimd SWDGE queue: big transfers where the
        # ~2us software setup cost is amortized, keeping the two HWDGE rings
        # free for the latency-critical startup slabs. Their triggers (and
        # bF's) are gated behind the first down-projection matmul so the
        # chunk-0 critical slabs monopolize HBM bandwidth first.
        late_dmas = [bf_dma]
        wts = [w0_t]
        for g in range(1, WG):
            wt_g = wt_pool.tile([P, KB, WGC], bf16)
            late_dmas.append(nc.gpsimd.dma_start(wt_g[:], wT[g]))
            wts.append(wt_g)

        def load_chunk(t, gate=None):
            xq_c = xq_pool.tile([P, KD, 2, TW], f8)
            xq_dma = nc.sync.dma_start(xq_c[:], xQ[t])
            xb_c = x_pool.tile([P, KB, TW], bf16)
            xb_dma = nc.sync.dma_start(xb_c[:], xT[t])
            if gate is not None:
                add_dep_helper(xq_dma.ins, gate.ins, sync=True, reason="pace")
                add_dep_helper(xb_dma.ins, gate.ins, sync=True, reason="pace")
            return (
                lambda kk, _x=xq_c: _x[:, kk, :, :],
                lambda kb, _x=xb_c: _x[:, kb, :],
            )

        def wslice(g, kb, loc):
            return wts[g][:, kb, loc : loc + P]

        def down(xq_slice, mk, after_first=None):
            """8 fp8 DoubleRow MMs + masked select; returns (um, first_mm)."""
            pu = pu_pool.tile([P, TW], mybir.dt.float32, tag="pu", name="pu")
            first = None
            for kk in range(KD):
                dmm = nc.tensor.matmul(
                    pu[:],
                    aq[:, kk, :, :],
                    xq_slice(kk),
                    start=(kk == 0),
                    stop=(kk == KD - 1),
                    perf_mode=DR,
                )
                if kk == 0:
                    first = dmm
                    if after_first is not None:
                        after_first()
            um = u_pool.tile([P, TW], bf16, tag="um", name="um")
            nc.vector.tensor_tensor(um[:], pu[:], mk[:], op=mybir.AluOpType.mult)
            return um, first

        def open_group(oi, xq_slice):
            # fp8 DoubleRow pairs cover contraction dims 0..256*KQ-1.
            po = po_pool.tile([P, TW], mybir.dt.float32)
            for kk in range(KQ):
                nc.tensor.matmul(
                    po[:],
                    wq[:, kk, :, oi * P : (oi + 1) * P],
                    xq_slice(kk),
                    start=(kk == 0),
                    stop=False,
                    perf_mode=DR,
                )
            return po

        def finish_group(t, oi, po, um):
            up = nc.tensor.matmul(
                po[:],
                bf_t[:, oi * P : (oi + 1) * P],
                um[:],
                start=False,
                stop=True,
            )
            ot = o_pool.tile([P, TW], mybir.dt.float32)
            nc.scalar.add(ot[:], po[:], bias_t[:, oi : oi + 1])
            nc.sync.dma_start(
                outT[oi * P : (oi + 1) * P, t * TW : (t + 1) * TW], ot[:]
            )
            return up

        # ---- chunk 0 solo: kb-outer quad 0, then quads 1..3 ----
        xq0_slice = lambda kk: xq0_t[:, kk, :, :]  # noqa: E731
        xb0_slice = lambda kb: x0_t[:, kb, :]  # noqa: E731

        quad0_po = []

        def _open_quad0():
            # Interleave quad-0's fp8 group openers into the DMA-paced
            # down-projection window.
            for oi in range(4):
                quad0_po.append(open_group(oi, xq0_slice))

        um0, down0_first = down(xq0_slice, mk0, after_first=_open_quad0)
        for dma in late_dmas:
            add_dep_helper(dma.ins, down0_first.ins, sync=True, reason="pace")

        # quad 0 @ t0: kb-outer so the PE consumes each k-slice on arrival.
        for kb in range(KB):
            for oi in range(4):
                nc.tensor.matmul(
                    quad0_po[oi],
                    wslice(0, kb, oi * P),
                    xb0_slice(kb),
                    start=False,
                    stop=False,
                )
        quad0_last_up = None
        for oi in range(4):
            quad0_last_up = finish_group(0, oi, quad0_po[oi], um0)
        for oi in range(4, OI):
            g = oi // WG
            loc = (oi % WG) * P
            po = open_group(oi, xq0_slice)
            for kb in range(KB):
                nc.tensor.matmul(
                    po[:], wslice(g, kb, loc), xb0_slice(kb), start=False, stop=False
                )
            finish_group(0, oi, po, um0)

        # ---- chunk 1 solo (loads gated until quad 0 is done so the W
        # groups get the early HBM bandwidth) ----
        xq1_slice, xb1_slice = load_chunk(1, gate=quad0_last_up)
        mk1 = load_mask(1, gate=quad0_last_up)
        um1, down1_first = down(xq1_slice, mk1)
        for oi in range(OI):
            g = oi // WG
            loc = (oi % WG) * P
            po = open_group(oi, xq1_slice)
            for kb in range(KB):
                nc.tensor.matmul(
                    po[:], wslice(g, kb, loc), xb1_slice(kb), start=False, stop=False
                )
            finish_group(1, oi, po, um1)

        # ---- steady-state pairs (2,3), (4,5), (6,7) ----
        for tp in range(2, TC, 2):
            gate = down1_first if tp == 2 else None
            xqa, xba = load_chunk(tp, gate=gate)
            xqb, xbb = load_chunk(tp + 1, gate=gate)
            mka = load_mask(tp, gate=gate)
            mkb = load_mask(tp + 1, gate=gate)
            um_a, _ = down(xqa, mka)
            um_b, _ = down(xqb, mkb)
            # Open each group one slot early: the next group's fp8
            # LDWEIGHTS prefetches under the current group's kb tail
            # instead of stalling behind the up-projections.
            po_a = open_group(0, xqa)
            po_b = open_group(0, xqb)
            for oi in range(OI):
                g = oi // WG
                loc = (oi % WG) * P
                for kb in range(KB):
                    ws = wslice(g, kb, loc)
                    nc.tensor.matmul(
                        po_a[:], ws, xba(kb), start=False, stop=False
                    )
                    nc.tensor.matmul(
                        po_b[:], ws, xbb(kb), start=False, stop=False
                    )
                cur_a, cur_b = po_a, po_b
                if oi + 1 < OI:
                    po_a = open_group(oi + 1, xqa)
                    po_b = open_group(oi + 1, xqb)
                finish_group(tp, oi, cur_a, um_a)
                finish_group(tp + 1, oi, cur_b, um_b)

    nc.compile()
    return nc


def _get_nc():
    if "nc" not in _CACHE:
        _CACHE["nc"] = _build()
    return _CACHE["nc"]


def _install_trace_shim():
    """This image's antenv lacks axon_hooks; register the NTFF profile hook
    ourselves so run_bass_kernel_spmd(trace=True) can capture exec_time_ns."""
    import sys
    import types

    if "antenv.axon_hooks" in sys.modules:
        return
    import antenv

    mod = types.ModuleType("antenv.axon_hooks")
    state = {"hook": None}
    mod.set_axon_ntff_profile_hook = lambda h: state.__setitem__("hook", h)
    mod.get_axon_ntff_profile_hook = lambda: state["hook"]
    sys.modules["antenv.axon_hooks"] = mod
    antenv.axon_hooks = mod

    from trn_agent_boot.trn_boot import _ntff_profile_via_ctypes

    mod.set_axon_ntff_profile_hook(
        _ntff_profile_via_ctypes("/opt/axon/libaxon_pjrt.so")
    )

    # No S3 in this container; keep artifacts local.
    import concourse.bass_utils as bu

    bu.upload_artifacts = lambda tmpdir: f"local://{tmpdir}"


def _q8(a):
    return np.clip(a, -240.0, 240.0).astype(_F8)


def kernel(x, W, b, A_all, B_all, lora_idx, _trace=False):
    global LAST_EXEC_TIME_NS
    from concourse.bass_utils import run_bass_kernel_spmd

    if _trace:
        try:
            _install_trace_shim()
        except Exception as e:  # degrade to untraced run
            print(f"trace shim failed ({e!r}); running untraced")
            _trace = False

    x = np.asarray(x, dtype=np.float32)
    W = np.asarray(W, dtype=np.float32)
    b = np.asarray(b, dtype=np.float32)
    A_all = np.asarray(A_all, dtype=np.float32)
    B_all = np.asarray(B_all, dtype=np.float32)
    lora_idx = np.asarray(lora_idx, dtype=np.int32)

    # Host-side weight reformat (replicated across cores), partition-major.
    w4 = W.reshape(WG, WGC, KT, P).transpose(0, 3, 2, 1)  # [g,p,k,o]
    wT_np = np.ascontiguousarray((w4[:, :, 2 * KQ :, :] * GS).astype(_BF16))
    wQ_np = np.ascontiguousarray(
        _q8(W[:, : 2 * KQ * P].reshape(D_OUT, KQ, 2, P) * SQ).transpose(3, 1, 2, 0)
    )
    A_flat = A_all.reshape(LR, D_IN)
    aQ_np = np.ascontiguousarray(
        _q8(A_flat.reshape(LR, KD, 2, P) * SQ).transpose(3, 1, 2, 0)
    )
    bF_np = np.ascontiguousarray(B_all.transpose(0, 2, 1)).reshape(LR, D_OUT).astype(
        _BF16
    )
    bias_np = np.ascontiguousarray((b * GS).reshape(OI, P).T).astype(np.float32)

    adapters = (np.arange(LR, dtype=np.int32) // R)[:, None]  # [LR, 1]

    in_maps = []
    for i in range(M_CORES):
        s = slice(i * NS, (i + 1) * NS)
        xr = x[s].reshape(TC, TW, KT, P)
        xT_i = np.ascontiguousarray(
            xr[:, :, 2 * KQ :, :].astype(_BF16).transpose(0, 3, 2, 1)
        )
        xQ_i = np.ascontiguousarray(
            _q8(xr.reshape(TC, TW, KD, 2, P) * SX).transpose(0, 4, 2, 3, 1)
        )
        idx = lora_idx[s]
        mfull = (adapters == idx[None, :]).astype(np.float32) * SCALE  # [LR, NS]
        mT_i = np.ascontiguousarray(
            mfull.astype(_BF16).reshape(LR, TC, TW).transpose(1, 0, 2)
        )
        in_maps.append(
            {
                "xT": xT_i,
                "xQ": xQ_i,
                "wT": wT_np,
                "wQ": wQ_np,
                "aQ": aQ_np,
                "bF": bF_np,
                "bias": bias_np,
                "mT": mT_i,
            }
        )

    nc = _get_nc()
    res = run_bass_kernel_spmd(
        nc, in_maps, core_ids=list(range(M_CORES)), trace=_trace
    )
    LAST_EXEC_TIME_NS = res.exec_time_ns

    out = np.empty((N, D_OUT), dtype=np.float32)
    inv = np.float32(1.0 / GS)
    for i in range(M_CORES):
        out[i * NS : (i + 1) * NS] = res.results[i]["outT"].T * inv
    return out


# revision 39
# speedup vs baseline: 1.0096x; 1.0096x over previous
"""Fused multi-LoRA linear layer on 8 TRN2 NeuronCores.

out = x @ W.T + b + scale * mask(x @ A_all^T) @ B_flat

Sharding: data-parallel over the token dim N (32768 -> 8 x 4096).
Weights (W, A_all, B_all, b) are replicated; each core computes its token
shard fully, so no collectives are needed.

v4: mixed-precision PE schedule + chunk-pair weight reuse.
- The LoRA down-projection and the first 256 contraction dims of the base
  matmul run as fp8e4 DoubleRow matmuls (2 contraction rows/cycle); the
  remaining 14 k-tiles stay bf16. All device math is scaled by GS = 2^16
  so fp8 and bf16 partial products share PSUM banks; the host descales.
- Token chunks are processed in pairs that share every stationary weight
  load (LDWEIGHTS gets a 2-matmul window to hide under).
- The first pair runs W-column-group-major so the PE starts ~2us after
  the first small DMAs land and each W group arrives before its quad.
"""

import numpy as np
import ml_dtypes

# Problem constants (hardcoded per harness contract).
N, D_IN, D_OUT, L, R = 32768, 2048, 2048, 8, 16
SCALE = 32.0 / 16.0
M_CORES = 8
NS = N // M_CORES  # 4096 tokens per core
P = 128
KT = D_IN // P  # 16 k-tiles
KQ = 2  # fp8 DoubleRow k-PAIRS used in the base matmul (k-tiles 0..2*KQ-1)
KB = KT - 2 * KQ  # bf16 k-tiles (k = 2*KQ .. 15)
KD = KT // 2  # fp8 k-pairs in the down-projection (all 8)
OI = D_OUT // P  # 16 output row-chunks of 128
TW = 512  # token tile width (moving free dim)
TC = NS // TW  # 8 token chunks per core
LR = L * R  # 128
WG = 4  # W column groups
WGC = D_OUT // WG  # 512 columns per group

SX = 32.0  # fp8 scale for x
SQ = 2048.0  # fp8 scale for W and A
GS = SX * SQ  # 2^16: global scale of all device-side math

_BF16 = ml_dtypes.bfloat16
_F8 = ml_dtypes.float8_e4m3

_CACHE = {}

LAST_EXEC_TIME_NS = None


def _build():
    import concourse.bass as bass  # noqa: F401
    import concourse.tile as tile
    from concourse import bacc, mybir
    from concourse.tile_rust import add_dep_helper
    from contextlib import ExitStack

    bf16 = mybir.dt.bfloat16
    f8 = mybir.dt.float8e4
    f32 = mybir.dt.float32
    DR = mybir.MatmulPerfMode.DoubleRow

    nc = bacc.Bacc(
        "TRN2",
        target_bir_lowering=False,
        debug=False,
        num_devices=M_CORES,
    )

    # Host-prepared, partition-major layouts (see kernel()):
    #   xT [TC, P, KB, TW] bf16 : xT[t,p,kb,j] = x[t*TW+j, (kb+2KQ)*P+p]
    #   xQ [TC, P, KD, 2, TW] f8: xQ[t,p,kk,u,j] = q8(x[t*TW+j, (2kk+u)*P+p]*SX)
    #   wT [WG, P, KB, WGC] bf16: wT[g,p,kb,o] = W[g*WGC+o, (kb+2KQ)*P+p]*GS
    #   wQ [P, KQ, 2, D_OUT] f8 : wQ[p,kk,u,o] = q8(W[o, (2kk+u)*P+p]*SQ)
    #   aQ [P, KD, 2, LR] f8    : aQ[p,kk,u,c] = q8(A_flat[c, (2kk+u)*P+p]*SQ)
    #   bF [P, D_OUT] bf16      : bF[c,o] = B_all[c//R, o, c%R]
    #   bias [P, OI] f32        : bias[p,oi] = b[oi*P+p]*GS
    #   mT [TC, P, TW] bf16     : one-hot adapter mask * SCALE
    xT = nc.dram_tensor("xT", [TC, P, KB, TW], bf16, kind="ExternalInput").ap()
    xQ = nc.dram_tensor("xQ", [TC, P, KD, 2, TW], f8, kind="ExternalInput").ap()
    wT = nc.dram_tensor("wT", [WG, P, KB, WGC], bf16, kind="ExternalInput").ap()
    wQ = nc.dram_tensor("wQ", [P, KQ, 2, D_OUT], f8, kind="ExternalInput").ap()
    aQ = nc.dram_tensor("aQ", [P, KD, 2, LR], f8, kind="ExternalInput").ap()
    bF = nc.dram_tensor("bF", [P, D_OUT], bf16, kind="ExternalInput").ap()
    bias = nc.dram_tensor("bias", [P, OI], f32, kind="ExternalInput").ap()
    mT = nc.dram_tensor("mT", [TC, P, TW], bf16, kind="ExternalInput").ap()
    outT = nc.dram_tensor("outT", [D_OUT, NS], f32, kind="ExternalOutput").ap()

    with tile.TileContext(nc) as tc, ExitStack() as ctx:
        warm_pool = ctx.enter_context(tc.tile_pool(name="warm", bufs=1))
        aq_pool = ctx.enter_context(tc.tile_pool(name="aq", bufs=1))
        wq_pool = ctx.enter_context(tc.tile_pool(name="wq", bufs=1))
        bf_pool = ctx.enter_context(tc.tile_pool(name="bfp", bufs=1))
        bias_pool = ctx.enter_context(tc.tile_pool(name="bias", bufs=1))
        mask_pool = ctx.enter_context(tc.tile_pool(name="mask", bufs=4))
        # chunk-0 slab-loaded input tiles (stay resident; the w0 tile serves
        # as the g=0 weight tile for every chunk)
        xq0_pool = ctx.enter_context(tc.tile_pool(name="xq0", bufs=1))
        x0_pool = ctx.enter_context(tc.tile_pool(name="x0", bufs=1))
        w0_pool = ctx.enter_context(tc.tile_pool(name="w0", bufs=1))
        wt_pool = ctx.enter_context(tc.tile_pool(name="wt", bufs=WG - 1))
        xq_pool = ctx.enter_context(tc.tile_pool(name="xq", bufs=4))
        x_pool = ctx.enter_context(tc.tile_pool(name="x", bufs=4))
        u_pool = ctx.enter_context(tc.tile_pool(name="u", bufs=2))
        o_pool = ctx.enter_context(tc.tile_pool(name="o", bufs=4))
        pw_pool = ctx.enter_context(tc.tile_pool(name="pw", bufs=1, space="PSUM"))
        pu_pool = ctx.enter_context(tc.tile_pool(name="pu", bufs=2, space="PSUM"))
        po_pool = ctx.enter_context(tc.tile_pool(name="po", bufs=4, space="PSUM"))

        # Short PE warmup: covers the first small DMAs and starts the HAM
        # clock ramp while chunk-0 slices stream in.
        warm = warm_pool.tile([P, P], bf16)
        nc.vector.memset(warm[:], 0.0)
        pw = pw_pool.tile([P, P], mybir.dt.float32)
        for _ in range(32):
            nc.tensor.matmul(pw[:], warm[:], warm[:], start=True, stop=True)

        def load_mask(t, gate=None):
            mk = mask_pool.tile([P, TW], bf16, tag="mk", name="mk")
            dma = nc.sync.dma_start(mk[:], mT[t])
            if gate is not None:
                add_dep_helper(dma.ins, gate.ins, sync=True, reason="pace")
            return mk

        # sync ring: small critical-path tensors, then chunk-0 slabs.
        aq = aq_pool.tile([P, KD, 2, LR], f8)
        nc.sync.dma_start(aq[:], aQ[:, :, :, :])
        xq0_t = xq0_pool.tile([P, KD, 2, TW], f8)
        for lo, hi in ((0, 3), (3, 6), (6, KD)):
            nc.sync.dma_start(xq0_t[:, lo:hi, :, :], xQ[0, :, lo:hi, :, :])
        mk0 = load_mask(0)
        bias_t = bias_pool.tile([P, OI], f32)
        nc.sync.dma_start(bias_t[:], bias[:, :])
        x0_t = x0_pool.tile([P, KB, TW], bf16)
        for lo in range(0, KB, 3):
            hi = min(lo + 3, KB)
            nc.sync.dma_start(x0_t[:, lo:hi, :], xT[0, :, lo:hi, :])

        # scalar ring: fp8 base weights, then g0 slabs.
        wq = wq_pool.tile([P, KQ, 2, D_OUT], f8)
        nc.scalar.dma_start(wq[:], wQ[:, :, :, :])
        w0_t = w0_pool.tile([P, KB, WGC], bf16)
        for lo in range(0, KB, 3):
            hi = min(lo + 3, KB)
            nc.scalar.dma_start(w0_t[:, lo:hi, :], wT[0, :, lo:hi, :])
        bf_t = bf_pool.tile([P, D_OUT], bf16)
        bf_dma = nc.scalar.dma_start(bf_t[:], bF[:, :])
        # g1..g3 ride the idle GpSimd SWDGE queue: big transfers where the
        # ~2us software setup cost is amortized, keeping the two HWDGE rings
        # free for the latency-critical startup slabs. Their triggers (and
        # bF's) are gated behind the first down-projection matmul so the
        # chunk-0 critical slabs monopolize HBM bandwidth first.
        late_dmas = [bf_dma]
        wts = [w0_t]
        for g in range(1, WG):
            wt_g = wt_pool.tile([P, KB, WGC], bf16)
            late_dmas.append(nc.gpsimd.dma_start(wt_g[:], wT[g]))
            wts.append(wt_g)

        def load_chunk(t, gate=None):
            xq_c = xq_pool.tile([P, KD, 2, TW], f8)
            xq_dma = nc.sync.dma_start(xq_c[:], xQ[t])
            xb_c = x_pool.tile([P, KB, TW], bf16)
            xb_dma = nc.sync.dma_start(xb_c[:], xT[t])
            if gate is not None:
                add_dep_helper(xq_dma.ins, gate.ins, sync=True, reason="pace")
                add_dep_helper(xb_dma.ins, gate.ins, sync=True, reason="pace")
            return (
                lambda kk, _x=xq_c: _x[:, kk, :, :],
                lambda kb, _x=xb_c: _x[:, kb, :],
            )

        def wslice(g, kb, loc):
            return wts[g][:, kb, loc : loc + P]

        def down(xq_slice, mk, after_first=None):
            """8 fp8 DoubleRow MMs + masked select; returns (um, first_mm)."""
            pu = pu_pool.tile([P, TW], mybir.dt.float32, tag="pu", name="pu")
            first = None
            for kk in range(KD):
                dmm = nc.tensor.matmul(
                    pu[:],
                    aq[:, kk, :, :],
                    xq_slice(kk),
                    start=(kk == 0),
                    stop=(kk == KD - 1),
                    perf_mode=DR,
                )
                if kk == 0:
                    first = dmm
                    if after_first is not None:
                        after_first()
            um = u_pool.tile([P, TW], bf16, tag="um", name="um")
            nc.vector.tensor_tensor(um[:], pu[:], mk[:], op=mybir.AluOpType.mult)
            return um, first

        def open_group(oi, xq_slice):
            # fp8 DoubleRow pairs cover contraction dims 0..256*KQ-1.
            po = po_pool.tile([P, TW], mybir.dt.float32)
            for kk in range(KQ):
                nc.tensor.matmul(
                    po[:],
                    wq[:, kk, :, oi * P : (oi + 1) * P],
                    xq_slice(kk),
                    start=(kk == 0),
                    stop=False,
                    perf_mode=DR,
                )
            return po

        def finish_group(t, oi, po, um):
            up = nc.tensor.matmul(
                po[:],
                bf_t[:, oi * P : (oi + 1) * P],
                um[:],
                start=False,
                stop=True,
            )
            ot = o_pool.tile([P, TW], mybir.dt.float32)
            nc.scalar.add(ot[:], po[:], bias_t[:, oi : oi + 1])
            nc.sync.dma_start(
                outT[oi * P : (oi + 1) * P, t * TW : (t + 1) * TW], ot[:]
            )
            return up

        # ---- chunk 0 solo: kb-outer quad 0, then quads 1..3 ----
        xq0_slice = lambda kk: xq0_t[:, kk, :, :]  # noqa: E731
        xb0_slice = lambda kb: x0_t[:, kb, :]  # noqa: E731

        quad0_po = []

        def _open_quad0():
            # Interleave quad-0's fp8 group openers into the DMA-paced
            # down-projection window.
            for oi in range(4):
                quad0_po.append(open_group(oi, xq0_slice))

        um0, down0_first = down(xq0_slice, mk0, after_first=_open_quad0)
        for dma in late_dmas:
            add_dep_helper(dma.ins, down0_first.ins, sync=True, reason="pace")

        # quad 0 @ t0: kb-outer so the PE consumes each k-slice on arrival.
        for kb in range(KB):
            for oi in range(4):
                nc.tensor.matmul(
                    quad0_po[oi],
                    wslice(0, kb, oi * P),
                    xb0_slice(kb),
                    start=False,
                    stop=False,
                )
        quad0_last_up = None
        for oi in range(4):
            quad0_last_up = finish_group(0, oi, quad0_po[oi], um0)
        for oi in range(4, OI):
            g = oi // WG
            loc = (oi % WG) * P
            po = open_group(oi, xq0_slice)
            for kb in range(KB):
                nc.tensor.matmul(
                    po[:], wslice(g, kb, loc), xb0_slice(kb), start=False, stop=False
                )
            finish_group(0, oi, po, um0)

        # ---- chunk 1 solo (loads gated until quad 0 is done so the W
        # groups get the early HBM bandwidth) ----
        xq1_slice, xb1_slice = load_chunk(1, gate=quad0_last_up)
        mk1 = load_mask(1, gate=quad0_last_up)
        um1, down1_first = down(xq1_slice, mk1)
        for oi in range(OI):
            g = oi // WG
            loc = (oi % WG) * P
            po = open_group(oi, xq1_slice)
            for kb in range(KB):
                nc.tensor.matmul(
                    po[:], wslice(g, kb, loc), xb1_slice(kb), start=False, stop=False
                )
            finish_group(1, oi, po, um1)

        # ---- steady-state pairs (2,3), (4,5), (6,7) ----
        for tp in range(2, TC, 2):
            gate = down1_first if tp == 2 else None
            xqa, xba = load_chunk(tp, gate=gate)
            xqb, xbb = load_chunk(tp + 1, gate=gate)
            mka = load_mask(tp, gate=gate)
            mkb = load_mask(tp + 1, gate=gate)
            um_a, _ = down(xqa, mka)
            um_b, _ = down(xqb, mkb)
            for oi in range(OI):
                g = oi // WG
                loc = (oi % WG) * P
                po_a = open_group(oi, xqa)
                po_b = open_group(oi, xqb)
                for kb in range(KB):
                    ws = wslice(g, kb, loc)
                    nc.tensor.matmul(
                        po_a[:], ws, xba(kb), start=False, stop=False
                    )
                    nc.tensor.matmul(
                        po_b[:], ws, xbb(kb), start=False, stop=False
                    )
                finish_group(tp, oi, po_a, um_a)
                finish_group(tp + 1, oi, po_b, um_b)

    nc.compile()
    return nc


def _get_nc():
    if "nc" not in _CACHE:
        _CACHE["nc"] = _build()
    return _CACHE["nc"]


def _install_trace_shim():
    """This image's antenv lacks axon_hooks; register the NTFF profile hook
    ourselves so run_bass_kernel_spmd(trace=True) can capture exec_time_ns."""
    import sys
    import types

    if "antenv.axon_hooks" in sys.modules:
        return
    import antenv

    mod = types.ModuleType("antenv.axon_hooks")
    state = {"hook": None}
    mod.set_axon_ntff_profile_hook = lambda h: state.__setitem__("hook", h)
    mod.get_axon_ntff_profile_hook = lambda: state["hook"]
    sys.modules["antenv.axon_hooks"] = mod
    antenv.axon_hooks = mod

    from trn_agent_boot.trn_boot import _ntff_profile_via_ctypes

    mod.set_axon_ntff_profile_hook(
        _ntff_profile_via_ctypes("/opt/axon/libaxon_pjrt.so")
    )

    # No S3 in this container; keep artifacts local.
    import concourse.bass_utils as bu

    bu.upload_artifacts = lambda tmpdir: f"local://{tmpdir}"


def _q8(a):
    return np.clip(a, -240.0, 240.0).astype(_F8)


def kernel(x, W, b, A_all, B_all, lora_idx, _trace=False):
    global LAST_EXEC_TIME_NS
    from concourse.bass_utils import run_bass_kernel_spmd

    if _trace:
        try:
            _install_trace_shim()
        except Exception as e:  # degrade to untraced run
            print(f"trace shim failed ({e!r}); running untraced")
            _trace = False

    x = np.asarray(x, dtype=np.float32)
    W = np.asarray(W, dtype=np.float32)
    b = np.asarray(b, dtype=np.float32)
    A_all = np.asarray(A_all, dtype=np.float32)
    B_all = np.asarray(B_all, dtype=np.float32)
    lora_idx = np.asarray(lora_idx, dtype=np.int32)

    # Host-side weight reformat (replicated across cores), partition-major.
    w4 = W.reshape(WG, WGC, KT, P).transpose(0, 3, 2, 1)  # [g,p,k,o]
    wT_np = np.ascontiguousarray((w4[:, :, 2 * KQ :, :] * GS).astype(_BF16))
    wQ_np = np.ascontiguousarray(
        _q8(W[:, : 2 * KQ * P].reshape(D_OUT, KQ, 2, P) * SQ).transpose(3, 1, 2, 0)
    )
    A_flat = A_all.reshape(LR, D_IN)
    aQ_np = np.ascontiguousarray(
        _q8(A_flat.reshape(LR, KD, 2, P) * SQ).transpose(3, 1, 2, 0)
    )
    bF_np = np.ascontiguousarray(B_all.transpose(0, 2, 1)).reshape(LR, D_OUT).astype(
        _BF16
    )
    bias_np = np.ascontiguousarray((b * GS).reshape(OI, P).T).astype(np.float32)

    adapters = (np.arange(LR, dtype=np.int32) // R)[:, None]  # [LR, 1]

    in_maps = []
    for i in range(M_CORES):
        s = slice(i * NS, (i + 1) * NS)
        xr = x[s].reshape(TC, TW, KT, P)
        xT_i = np.ascontiguousarray(
            xr[:, :, 2 * KQ :, :].astype(_BF16).transpose(0, 3, 2, 1)
        )
        xQ_i = np.ascontiguousarray(
            _q8(xr.reshape(TC, TW, KD, 2, P) * SX).transpose(0, 4, 2, 3, 1)
        )
        idx = lora_idx[s]
        mfull = (adapters == idx[None, :]).astype(np.float32) * SCALE  # [LR, NS]
        mT_i = np.ascontiguousarray(
            mfull.astype(_BF16).reshape(LR, TC, TW).transpose(1, 0, 2)
        )
        in_maps.append(
            {
                "xT": xT_i,
                "xQ": xQ_i,
                "wT": wT_np,
                "wQ": wQ_np,
                "aQ": aQ_np,
                "bF": bF_np,
                "bias": bias_np,
                "mT": mT_i,
            }
        )

    nc = _get_nc()
    res = run_bass_kernel_spmd(
        nc, in_maps, core_ids=list(range(M_CORES)), trace=_trace
    )
    LAST_EXEC_TIME_NS = res.exec_time_ns

    out = np.empty((N, D_OUT), dtype=np.float32)
    inv = np.float32(1.0 / GS)
    for i in range(M_CORES):
        out[i * NS : (i + 1) * NS] = res.results[i]["outT"].T * inv
    return out
